# revision 53
# baseline (speedup 1.0000x reference)
"""BloomAttention (B=1, S=2048, HID=4096, NH=32) on 8 Trainium2 NeuronCores.

v2 strategy (tensor-parallel over heads):
  - Heads assigned per core by octile slots: core c owns global heads
    {c, c+8, c+16, c+24}. Slot k's ALiBi slope is at most 2^-(2k+2), so
    attention blocks farther than D_slot = 30/slope_min from the causal
    diagonal contribute < ~1e-7 relative mass and are skipped (structure is
    baked uniformly across cores; slope VALUES stay per-core runtime data).
  - hidden is transposed on HOST (hT [HID, S]) - zero on-device transposes.
  - Flash order: per 512-quarter, QKV matmuls then attention. q/k produced
    feature-major (weights stationary); V produced sequence-major directly
    (hT blocks stationary, w_v moving) so PV needs no transpose.
  - Scores in transposed layout [sk, sq]: alibi+causal+per-query shift via a
    single wide masked distance table T[a,c] = (a-c <= 0 ? a-c : -60000),
    applied by one DVE scalar_tensor_tensor per (column-striped) block; exp
    on ACT (one act table load total); denominator via ones[128,128]
    stationary matmuls accumulating a broadcast row-sum in PSUM; 1/d via
    nc.vector.reciprocal_approx_fast; V-bias folded into b_dense on host.
  - Four AllToAlls (0.5MB, one per local head) swap head-shards for
    sequence-shards; heads 0,1 fire while heads 2,3 still compute. Dense:
    w_dense streamed in [16 ft x 1024 o] chunks, crecv stationary (LDW
    amortized over the two 512-o halves), first chunk split into four
    ft%4 passes so each pass starts as its collective lands; bias added
    via hi/lo bf16 ones-matmuls (exact to ~2^-17).
"""

import math
import os
import sys
import types
from contextlib import ExitStack

import numpy as np
import ml_dtypes

B, S, HID, NH, HD = 1, 2048, 4096, 32, 128
NCORES = 8
NH_LOC = NH // NCORES            # 4 heads per core (slots)
SROW = S // NCORES               # 256 output rows per core
INV_NORM = 1.0 / math.sqrt(HD)
KT = HID // HD                   # 32 k tiles
TW = 2432                        # wide distance-table columns
NEG = -60000.0
DSLOT = [120, 480, 1920, 2048]   # per-slot causal stripe depth (30/slope_min)

_CACHE = {}


def _ensure_axon_hooks():
    try:
        import antenv  # noqa: F401

        extra = "/opt/trn_rl_repo/antenv"
        if os.path.isdir(extra) and extra not in antenv.__path__:
            antenv.__path__.append(extra)
        import antenv.axon_hooks  # noqa: F401
    except Exception:
        hook = None
        try:
            from trn_agent_boot.trn_boot import _ntff_profile_via_ctypes

            hook = _ntff_profile_via_ctypes("/opt/axon/libaxon_pjrt.so")
        except Exception:
            hook = None
        m = types.ModuleType("antenv.axon_hooks")
        m._hook = hook
        m.get_axon_ntff_profile_hook = lambda: m._hook
        m.set_axon_ntff_profile_hook = lambda h: setattr(m, "_hook", h)
        sys.modules["antenv.axon_hooks"] = m


def _surv(hl, q):
    """Surviving (skt, vs0, ve) column stripes for local head hl, quarter q.

    First surviving block is widened to the full 512 columns so its
    start=True matmul initialises every PSUM column of ps_ctx / ps_bc.
    """
    D = DSLOT[hl]
    sq0 = 512 * q
    out = []
    for skt in range(4 * q + 4):
        vs0 = max(0, 128 * skt - sq0)
        ve = min(512, 128 * skt + 128 + D - sq0)
        if ve <= vs0:
            continue
        out.append([skt, vs0, ve])
    out[0][1] = 0
    out[0][2] = 512
    return out


def _build_nc():
    import concourse.bass as bass  # noqa: F401
    import concourse.mybir as mybir
    from concourse import bacc, tile

    BF = mybir.dt.bfloat16
    F16 = mybir.dt.float16
    F32 = mybir.dt.float32
    Alu = mybir.AluOpType
    Act = mybir.ActivationFunctionType

    nc = bacc.Bacc(None, target_bir_lowering=False, num_devices=NCORES)
    with tile.TileContext(nc) as tc, ExitStack() as ctx:
        dram = ctx.enter_context(tc.tile_pool(name="dram", bufs=1, space="DRAM"))

        def din(name, shape, dt):
            return dram.tile(shape, dt, kind="ExternalInput", name=name,
                             uniquify=False)

        hTd = din("hT", [HD, KT, S], BF)
        wqkd = din("wqk", [HD, KT, 8 * HD], BF)
        wvd = din("wv", [HD, KT, 4 * HD], BF)
        bqkd = din("bqk", [HD, 8], F32)
        tmatd = din("tmat", [HD, TW], F16)
        ones16d = din("ones16", [HD, HD], BF)
        slopesd = din("slopes", [HD, NH_LOC], F32)
        wdd = din("wd", [8, HD, 16, 1024], BF)
        bdhd = din("bdh", [1, HID], BF)
        bdld = din("bdl", [1, HID], BF)
        out = dram.tile([SROW, HID], F32, kind="ExternalOutput", name="out",
                        uniquify=False)
        a2a_in = [dram.tile([NCORES, HD, SROW], BF, name=f"a2a_in{p}")
                  for p in range(NH_LOC)]
        a2a_out = [dram.tile([NCORES, HD, SROW], BF, name=f"a2a_out{p}")
                   for p in range(NH_LOC)]

        # ---------- persistent SBUF ----------
        const = ctx.enter_context(tc.tile_pool(name="const", bufs=1))
        sb_bqk = const.tile([HD, 8], F32)
        nc.scalar.dma_start(out=sb_bqk[:], in_=bqkd[:])
        sb_slopes = const.tile([HD, NH_LOC], F32)
        nc.scalar.dma_start(out=sb_slopes[:], in_=slopesd[:])
        tmat = const.tile([HD, TW], F16)
        nc.scalar.dma_start(out=tmat[:], in_=tmatd[:])
        ones128 = const.tile([HD, HD], BF)
        nc.scalar.dma_start(out=ones128[:], in_=ones16d[:])
        ones1 = const.tile([1, HD], BF)
        nc.vector.memset(ones1[:], 1.0)


        persist = ctx.enter_context(tc.tile_pool(name="persist", bufs=1))
        kT = [persist.tile([HD, S], BF, name=f"kT{h}") for h in range(NH_LOC)]
        vnat = persist.tile([HD, 16, 4 * HD], BF)  # [p, sb, hl*128+d]
        qT = persist.tile([HD, NH_LOC, 512], BF)    # current quarter only

        # attention pools (open for the whole run)
        expp = ctx.enter_context(tc.tile_pool(name="expp", bufs=4))
        recp = ctx.enter_context(tc.tile_pool(name="recp", bufs=2))
        ctxp = ctx.enter_context(tc.tile_pool(name="ctxp", bufs=2))
        sc_ps = ctx.enter_context(
            tc.tile_pool(name="sc_ps", bufs=2, space="PSUM"))
        ctx_ps = ctx.enter_context(
            tc.tile_pool(name="ctx_ps", bufs=1, space="PSUM"))
        bc_ps = ctx.enter_context(
            tc.tile_pool(name="bc_ps", bufs=1, space="PSUM"))


        def attention(q, hls):
            q0 = 512 * q
            for hl in hls:
                slope = sb_slopes[:, hl:hl + 1]
                sl = _surv(hl, q)
                ps_ctx = ctx_ps.tile([HD, 512], F32, name="ps_ctx")
                ps_bc = bc_ps.tile([HD, 512], F32, name="ps_bc")
                exs = {}

                def flush(i, first, last):
                    ex, skt, vs0, ve = exs.pop(i)
                    nc.tensor.matmul(
                        ps_ctx[:, vs0:ve],
                        vnat[:, skt, hl * HD:(hl + 1) * HD],
                        ex[:, vs0:ve], start=first, stop=last)
                    nc.tensor.matmul(
                        ps_bc[:, vs0:ve], ones128[:],
                        ex[:, vs0:ve], start=first, stop=last)

                for i, (skt, vs0, ve) in enumerate(sl):
                    o = skt - 4 * q
                    ps = sc_ps.tile([HD, 512], F32, name="ps_sc")
                    nc.tensor.matmul(
                        ps[:, vs0:ve],
                        kT[hl][:, skt * HD:(skt + 1) * HD],
                        qT[:, hl, vs0:ve], start=True, stop=True)
                    c0 = vs0 - o * HD + 384
                    nc.vector.scalar_tensor_tensor(
                        ps[:, vs0:ve], tmat[:, c0:c0 + (ve - vs0)], slope,
                        ps[:, vs0:ve], Alu.mult, Alu.add)
                    ex = expp.tile([HD, 512], BF, name="ex")
                    nc.scalar.activation(ex[:, vs0:ve], ps[:, vs0:ve], Act.Exp)
                    exs[i] = (ex, skt, vs0, ve)
                    if i >= 2:
                        flush(i - 2, i - 2 == 0, False)
                n = len(sl)
                for i in (n - 2, n - 1):
                    if i >= 0 and i in exs:
                        flush(i, i == 0, i == n - 1)

                rec = recp.tile([HD, 512], F32, name="rec")
                nc.vector.reciprocal_approx_fast(rec[:], ps_bc[:])
                csb = ctxp.tile([HD, 512], BF, name="csb")
                nc.vector.tensor_tensor(csb[:], ps_ctx[:], rec[:], Alu.mult)
                for j in (0, 1):
                    nc.sync.dma_start(
                        out=a2a_in[hl][2 * q + j],
                        in_=csb[:, j * SROW:(j + 1) * SROW])
                if q == 3:
                    nc.gpsimd.collective_compute(
                        "AllToAll", Alu.bypass,
                        replica_groups=[list(range(NCORES))],
                        ins=[a2a_in[hl][:]], outs=[a2a_out[hl][:]])

        # ---------- phase 1: QKV + attention, interleaved per quarter ----
        with (
            tc.tile_pool(name="hT_pool", bufs=2) as hT_pool,
            tc.tile_pool(name="wqk_pool", bufs=8) as wqk_pool,
            tc.tile_pool(name="wv_pool", bufs=1) as wv_pool,
            tc.tile_pool(name="qkv_ps", bufs=1, space="PSUM") as qkv_ps,
        ):
            for q in range(4):
                ht = hT_pool.tile([HD, KT, 512], BF, name="ht")
                wv = wv_pool.tile([HD, KT, 4 * HD], BF, name="wv")
                for cg in range(4):
                    nc.scalar.dma_start(
                        out=wv[:, cg * 8:(cg + 1) * 8, :],
                        in_=wvd[:, cg * 8:(cg + 1) * 8, :])

                def qk_sweep(grp, wqs=None):
                    psl = [qkv_ps.tile([HD, 512], F32, name=f"qk{i}", bufs=1)
                           for i in range(4)]
                    for kt in range(KT):
                        if kt % 4 == 0:
                            if wqs is not None:
                                wq = wqs[kt // 4]
                            else:
                                wq = wqk_pool.tile([HD, 4, 4 * HD], BF,
                                                   name="wq")
                                nc.sync.dma_start(
                                    out=wq[:],
                                    in_=wqkd[:, kt:kt + 4,
                                             grp * 512:(grp + 1) * 512])
                        if grp == 0 and kt in (0, 4, 8, 16, 24):
                            k0 = kt if kt < 8 else kt
                            kn = 4 if kt < 8 else 8
                            nc.sync.dma_start(
                                out=ht[:, k0:k0 + kn, :],
                                in_=hTd[:, k0:k0 + kn,
                                        512 * q:512 * q + 512])
                        for i in range(4):
                            nc.tensor.matmul(
                                psl[i][:],
                                wq[:, kt % 4, i * HD:(i + 1) * HD],
                                ht[:, kt, :],
                                start=(kt == 0), stop=(kt == KT - 1))
                    for i in range(4):
                        hl = grp * 2 + i // 2
                        isq = i % 2 == 0
                        f = hl * 2 + (0 if isq else 1)
                        if isq:
                            dest = qT[:, hl, :]
                        else:
                            dest = kT[hl][:, 512 * q:512 * q + 512]
                        nc.scalar.activation(
                            dest, psl[i][:], Act.Identity,
                            bias=sb_bqk[:, f:f + 1])

                qk_sweep(0)
                # V sweep: natural layout, hT blocks stationary
                for sb in range(4):
                    psv = sc_ps.tile([HD, 512], F32, name="ps_sc")
                    for kt in range(KT):
                        nc.tensor.matmul(
                            psv[:], ht[:, kt, sb * HD:(sb + 1) * HD],
                            wv[:, kt, :], start=(kt == 0), stop=(kt == KT - 1))
                    nc.scalar.copy(vnat[:, 4 * q + sb, :], psv[:])
                wqs1 = None
                if q == 3:
                    # pull grp1's weight streams ahead of the collectives
                    wqs1 = []
                    for ci in range(8):
                        w = wqk_pool.tile([HD, 4, 4 * HD], BF, name="wq")
                        nc.sync.dma_start(
                            out=w[:],
                            in_=wqkd[:, ci * 4:ci * 4 + 4, 512:1024])
                        wqs1.append(w)
                attention(q, [0, 1])
                qk_sweep(1, wqs1)
                if q < 3:
                    attention(q, [2, 3])

        # ---------- phase 2: last attention heads + dense ----------
        with (
            tc.tile_pool(name="wd_pool", bufs=2) as wd_pool,
            tc.tile_pool(name="dns_sb", bufs=1) as dns_sb,
            tc.tile_pool(name="osb_pool", bufs=4) as osb_pool,
            tc.tile_pool(name="dns_ps", bufs=2, space="PSUM") as dns_ps,
        ):
            wd0 = wd_pool.tile([HD, 16, 1024], BF, name="wd")
            nc.scalar.dma_start(out=wd0[:], in_=wdd[0])
            attention(3, [2, 3])
            sb_bdh = dns_sb.tile([1, HID], BF)
            nc.scalar.dma_start(out=sb_bdh[:], in_=bdhd[:])
            sb_bdl = dns_sb.tile([1, HID], BF)
            nc.scalar.dma_start(out=sb_bdl[:], in_=bdld[:])
            crecv = dns_sb.tile([HD, KT, SROW], BF)
            for hl in range(NH_LOC):
                nc.scalar.dma_start(
                    out=crecv[:, hl:KT:NH_LOC, :],
                    in_=a2a_out[hl].rearrange("i p s -> p i s"))
            for oc in range(4):
                psd = [[dns_ps.tile([HD, 512], F32, name=f"psd{st}{oh}",
                                    bufs=1) for oh in range(2)]
                       for st in range(2)]
                first = True
                for ftc in range(2):
                    f0 = ftc * 16
                    wdc = wd0 if oc == 0 and ftc == 0 else wd_pool.tile(
                        [HD, 16, 1024], BF, name="wd")
                    if oc > 0 or ftc > 0:
                        nc.scalar.dma_start(out=wdc[:], in_=wdd[oc * 2 + ftc])
                    if oc == 0 and ftc == 0:
                        passes = [list(range(m, 16, NH_LOC))
                                  for m in range(NH_LOC)]
                    else:
                        passes = [list(range(f0, f0 + 16))]
                    for ftset in passes:
                        for st in range(2):
                            for ft in ftset:
                                for oh in range(2):
                                    nc.tensor.matmul(
                                        psd[st][oh][:],
                                        crecv[:, ft, st * HD:(st + 1) * HD],
                                        wdc[:, ft - f0,
                                            oh * 512:(oh + 1) * 512],
                                        start=first and ft == ftset[0],
                                        stop=False)
                        first = False
                for st in range(2):
                    for oh in range(2):
                        o0 = oc * 1024 + oh * 512
                        nc.tensor.matmul(psd[st][oh][:], ones1[:],
                                         sb_bdh[:, o0:o0 + 512],
                                         start=False, stop=False)
                        nc.tensor.matmul(psd[st][oh][:], ones1[:],
                                         sb_bdl[:, o0:o0 + 512],
                                         start=False, stop=True)
                        osb = osb_pool.tile([HD, 512], F32, name="osb")
                        nc.scalar.copy(osb[:], psd[st][oh][:])
                        nc.sync.dma_start(
                            out=out[st * HD:(st + 1) * HD, o0:o0 + 512],
                            in_=osb[:])
    nc.compile()
    return nc


def _prep_shards(hidden_states, alibi, w_qkv, b_qkv, w_dense, b_dense):
    bf16 = ml_dtypes.bfloat16
    hidden = np.asarray(hidden_states, dtype=np.float32).reshape(S, HID)
    hT = np.ascontiguousarray(hidden.T).astype(bf16)       # [HID, S]
    hTd = np.ascontiguousarray(hT.reshape(KT, HD, S).transpose(1, 0, 2))
    al = np.asarray(alibi, dtype=np.float32).reshape(NH, S)
    w = np.asarray(w_qkv, dtype=np.float32)                # [3H, H]
    b = np.asarray(b_qkv, dtype=np.float32)
    wd = np.asarray(w_dense, dtype=np.float32)             # [H, H]
    bd = np.asarray(b_dense, dtype=np.float32)

    wT = np.ascontiguousarray(w.T)                         # [H, 3H]

    # fold v-bias into dense bias: out = wd @ (ctx + bv) + bd
    bv_full = np.zeros(HID, np.float32)
    for g in range(NH):
        bv_full[g * HD:(g + 1) * HD] = b[g * 3 * HD + 2 * HD:
                                         g * 3 * HD + 3 * HD]
    bdf = bd + wd @ bv_full
    bdh = bdf.astype(bf16)
    bdl = (bdf - bdh.astype(np.float32)).astype(bf16)

    # wide masked distance table  T[a, c'] = a-c if a<=c else NEG, c=c'-384
    a = np.arange(HD)[:, None]
    cp = np.arange(TW)[None, :] - 384
    tmat = np.where(a <= cp, (a - cp).astype(np.float32), np.float32(NEG))
    tmat = tmat.astype(ml_dtypes.float16 if False else np.float16)

    in_maps = []
    for c in range(NCORES):
        heads = [c + 8 * hl for hl in range(NH_LOC)]
        # q/k weights, feature-major [p, kt, (hl, qk, d)]
        wqk = np.empty((KT, HD, 8 * HD), np.float32)
        wv = np.empty((KT, HD, 4 * HD), np.float32)
        bqk = np.empty((HD, 8), np.float32)
        for hl, g in enumerate(heads):
            r = g * 3 * HD
            wqk[:, :, hl * 2 * HD:hl * 2 * HD + HD] = \
                (wT[:, r:r + HD] * INV_NORM).reshape(KT, HD, HD)
            wqk[:, :, hl * 2 * HD + HD:(hl + 1) * 2 * HD] = \
                wT[:, r + HD:r + 2 * HD].reshape(KT, HD, HD)
            wv[:, :, hl * HD:(hl + 1) * HD] = \
                wT[:, r + 2 * HD:r + 3 * HD].reshape(KT, HD, HD)
            bqk[:, hl * 2] = b[r:r + HD] * INV_NORM
            bqk[:, hl * 2 + 1] = b[r + HD:r + 2 * HD]
        slopes = np.repeat(al[heads, 1:2].T, HD, axis=0)   # [128, 4]

        # dense weights: rows by global head of ft = i*4 + 2p + j,
        # g(ft) = 8*(ft%4) + ft//4 ; o-chunks of 512
        wdT = wd.T                                         # [f, o]
        wdr4 = np.empty((4, HD, KT, 1024), np.float32)
        for ft in range(KT):
            g = 8 * (ft % 4) + ft // 4
            blk = wdT[g * HD:(g + 1) * HD]                 # [128, 4096]
            wdr4[:, :, ft, :] = blk.reshape(HD, 4, 1024).transpose(1, 0, 2)
        wdr = wdr4.reshape(4, HD, 2, 16, 1024).transpose(
            0, 2, 1, 3, 4).reshape(8, HD, 16, 1024)

        in_maps.append({
            "hT": hTd,
            "wqk": np.ascontiguousarray(
                wqk.transpose(1, 0, 2)).astype(bf16),
            "wv": np.ascontiguousarray(wv.transpose(1, 0, 2)).astype(bf16),
            "bqk": np.ascontiguousarray(bqk),
            "tmat": tmat,
            "ones16": np.ones((HD, HD), bf16),
            "slopes": np.ascontiguousarray(slopes.astype(np.float32)),
            "wd": np.ascontiguousarray(wdr).astype(bf16),
            "bdh": bdh.reshape(1, HID),
            "bdl": bdl.reshape(1, HID),
        })
    return in_maps


def _unshard(res):
    # out rows of core c are s in [c*256, (c+1)*256)
    outp = np.concatenate([res.results[c]["out"] for c in range(NCORES)],
                          axis=0)
    return outp.reshape(B, S, HID).astype(np.float32)


def kernel(hidden_states, alibi, w_qkv, b_qkv, w_dense, b_dense):
    _ensure_axon_hooks()
    from concourse import bass_utils

    if "nc" not in _CACHE:
        _CACHE["nc"] = _build_nc()
    nc = _CACHE["nc"]
    in_maps = _prep_shards(hidden_states, alibi, w_qkv, b_qkv,
                           w_dense, b_dense)
    trace = bool(os.environ.get("BLOOM_TRACE"))
    res = bass_utils.run_bass_kernel_spmd(
        nc, in_maps, core_ids=list(range(NCORES)), trace=trace)
    kernel._last_results = res
    kernel._last_exec_ns = res.exec_time_ns
    return _unshard(res)
